# revision 9
# baseline (speedup 1.0000x reference)
"""Self-contained Trainium kernel for nn_Encoder_35682588295656.

Data-parallel over batch across 8 NeuronCores (4 batch blocks/core).
Structural optimizations vs the reference:
  - The block-diagonal graph is identical for every batch element (same
    local edge list, np.tile'd weights), so the sparse GraphConv segment-sum
    becomes ONE shared dense [1000x1000] bf16 matmul operand shipped once
    (checked at runtime; falls back to per-batch adjacency otherwise).
  - FC concat is split into h @ W_fc_h + x @ W_fc_x + bias_fc[t]; the
    pos-table contribution folds into the per-timestep bias on the host.
  - Wo/W_mlp fused into one projection; 1/sqrt(d) folded into Wq.
  - Attention runs per-head on an n-major layout so no head transposes or
    z-transpose are needed; softmax skips the max-subtraction (logits O(1)).
  - 16-bit wire formats (bf16 in / f16 out); f32 accumulation on device.
  - Adjacency + weights are cached device-resident across calls.
"""

import numpy as np

B, T_TOT, T, N, F, HID, EMB, HEADS, DEG = 32, 48, 24, 1000, 16, 64, 8, 4, 16
C = F + 1
D_HEAD = HID // HEADS
M = 8           # cores
BL = B // M     # batches per core

_cache = {}


def _build_shared_A(edge_src, edge_dst, edge_weight):
    es = np.asarray(edge_src)
    ed = np.asarray(edge_dst)
    w = np.asarray(edge_weight, np.float32)
    E = es.shape[0]
    if E % B:
        return None, False
    per = E // B
    es2 = es.reshape(B, per)
    ed2 = ed.reshape(B, per)
    w2 = w.reshape(B, per)
    blocks = (np.arange(B, dtype=es2.dtype) * N)[:, None]
    src_l = es2 - blocks
    dst_l = ed2 - blocks
    shared = bool(
        np.all(src_l[1:] == src_l[0]) and np.all(dst_l[1:] == dst_l[0])
        and np.all(w2[1:] == w2[0])
        and src_l.min() >= 0 and src_l.max() < N
        and dst_l.min() >= 0 and dst_l.max() < N
    )
    if not shared:
        return None, False
    A = np.zeros((N, N), np.float32)
    np.add.at(A, (dst_l[0], src_l[0]), w2[0])
    return A, True


def _build_A_all(edge_src, edge_dst, edge_weight):
    es = np.asarray(edge_src, np.int64)
    ed = np.asarray(edge_dst, np.int64)
    w = np.asarray(edge_weight, np.float32)
    b_idx = ed // N
    dst_l = ed - b_idx * N
    src_l = es - b_idx * N
    ok = (src_l >= 0) & (src_l < N)
    A = np.zeros((B, N, N), np.float32)
    np.add.at(A, (b_idx[ok], dst_l[ok], src_l[ok]), w[ok])
    return A


def _prep_weights(p):
    W_fc = np.asarray(p['W_fc'], np.float32)
    pos = np.asarray(p['pos_table'], np.float32)
    out = {}
    out['W_rel'] = np.asarray(p['W_rel'], np.float32)
    out['W_root'] = np.asarray(p['W_root'], np.float32)
    out['b_rel'] = np.asarray(p['b_rel'], np.float32)
    out['W_fc_h'] = W_fc[EMB:EMB + HID]
    out['W_fc_x'] = np.concatenate([W_fc[EMB + HID + F:EMB + HID + F + 1],
                                    W_fc[EMB + HID:EMB + HID + F]], 0)
    out['bias_fc'] = pos @ W_fc[:EMB] + np.asarray(p['b_fc'], np.float32)
    sc = 1.0 / np.sqrt(np.float32(D_HEAD))
    out['Wq'] = np.asarray(p['Wq'], np.float32) * sc
    out['bq'] = np.asarray(p['bq'], np.float32) * sc
    for n in ('Wk', 'bk', 'Wv', 'bv'):
        out[n] = np.asarray(p[n], np.float32)
    Wo = np.asarray(p['Wo'], np.float32)
    Wm = np.asarray(p['W_mlp'], np.float32)
    out['Wom'] = Wo @ Wm
    out['bom'] = np.asarray(p['bo'], np.float32) @ Wm + np.asarray(p['b_mlp'], np.float32)
    return out


def _enable_comp_cache():
    if _cache.get('cc'):
        return
    _cache['cc'] = True
    try:
        import jax
        jax.config.update('jax_compilation_cache_dir', '/tmp/jax_comp_cache')
        jax.config.update('jax_persistent_cache_min_compile_time_secs', 1.0)
        jax.config.update('jax_persistent_cache_min_entry_size_bytes', 0)
    except Exception:
        pass


def _get_fn(shared):
    key = ('fn', shared)
    if key in _cache:
        return _cache[key]
    import jax
    import jax.numpy as jnp
    _enable_comp_cache()

    f32 = jnp.float32
    bf = jnp.bfloat16

    def mm(a, b):
        return jax.lax.dot_general(a, b, (((a.ndim - 1,), (0,)), ((), ())),
                                   preferred_element_type=f32)

    def shard_fn(A, xw, W):
        # A: [N,N] bf16 (or [BL,N,N]); xw: [N, BL*T*C] bf16 node-major wire
        x = xw.reshape(N, BL, T, C).transpose(1, 0, 2, 3)       # [BL,N,T,C]
        if shared:
            agg = jax.lax.dot(A, xw, preferred_element_type=f32)
            agg = agg.reshape(N, BL, T, C).transpose(1, 0, 2, 3).astype(bf)
        else:
            agg = jnp.einsum('bij,bjtc->bitc', A,
                             xw.reshape(N, BL, T, C).transpose(1, 0, 2, 3),
                             preferred_element_type=f32).astype(bf)
        h = jax.nn.sigmoid(mm(agg, W['W_rel'].astype(bf))
                           + mm(x, W['W_root'].astype(bf)) + W['b_rel'])
        h = h.astype(bf)
        fc = (mm(h, W['W_fc_h'].astype(bf)) + mm(x, W['W_fc_x'].astype(bf))
              + W['bias_fc'][None, None, :, :])
        z = fc.astype(bf).reshape(BL * N, T, HID)
        ctxs = []
        for hi in range(HEADS):
            sl = slice(hi * D_HEAD, (hi + 1) * D_HEAD)
            qh = (mm(z, W['Wq'][:, sl].astype(bf)) + W['bq'][sl]).astype(bf)
            kh = (mm(z, W['Wk'][:, sl].astype(bf)) + W['bk'][sl]).astype(bf)
            vh = (mm(z, W['Wv'][:, sl].astype(bf)) + W['bv'][sl]).astype(bf)
            s = jax.lax.dot_general(qh, kh, (((2,), (2,)), ((0,), (0,))),
                                    preferred_element_type=f32)
            e = jnp.exp(s)
            a = (e / e.sum(-1, keepdims=True)).astype(bf)
            ctxs.append(jax.lax.dot_general(a, vh, (((2,), (1,)), ((0,), (0,))),
                                            preferred_element_type=f32).astype(bf))
        ctx = jnp.concatenate(ctxs, -1)
        o = mm(ctx, W['Wom'].astype(bf)) + W['bom']
        o = o.reshape(BL, N, T, HID).transpose(0, 2, 1, 3)
        return o.astype(jnp.float16)

    fn = jax.pmap(shard_fn, devices=jax.devices()[:M], in_axes=(0, 0, 0))
    _cache[key] = fn
    return fn


def _get_pool():
    if 'pool' not in _cache:
        from concurrent.futures import ThreadPoolExecutor
        _cache['pool'] = ThreadPoolExecutor(M)
    return _cache['pool']


def _stage_constants(A_np, W, shared):
    """Device-put A and weights once; reuse across calls with identical values."""
    import jax
    dev = jax.devices()[:M]
    c = _cache.get('const')
    if c is not None and c['shared'] == shared and np.array_equal(c['A_np'], A_np) \
            and all(np.array_equal(c['W_np'][k], W[k]) for k in W):
        return c['dA'], c['dW']
    import ml_dtypes
    bf16 = ml_dtypes.bfloat16
    pool = _get_pool()
    if shared:
        Ab = A_np.astype(bf16)
        futs = [pool.submit(jax.device_put, Ab, d) for d in dev]
        dA = jax.device_put_sharded([f.result() for f in futs], dev)
    else:
        Ab = A_np.astype(bf16).reshape(M, BL, N, N)
        futs = [pool.submit(jax.device_put, Ab[i], dev[i]) for i in range(M)]
        dA = jax.device_put_sharded([f.result() for f in futs], dev)
    dW = {k: jax.device_put_replicated(np.asarray(v), dev) for k, v in W.items()}
    _cache['const'] = dict(shared=shared, A_np=A_np, W_np={k: np.array(v) for k, v in W.items()},
                           dA=dA, dW=dW)
    return dA, dW


def _pack_x(X, y):
    import ml_dtypes
    bf16 = ml_dtypes.bfloat16
    # node-major wire layout [M, N, BL*T*C]
    xt = np.empty((M, N, BL, T, C), dtype=bf16)
    Xr = np.asarray(X)[:, :T].reshape(M, BL, T, N, F)
    yr = np.asarray(y)[:, :T].reshape(M, BL, T, N, 1)
    xt[..., 0] = yr[..., 0].transpose(0, 3, 1, 2).astype(bf16)
    xt[..., 1:] = Xr.transpose(0, 3, 1, 2, 4).astype(bf16)
    return xt.reshape(M, N, BL * T * C)


def kernel(X, y, edge_src, edge_dst, edge_weight, pos_table, W_rel, b_rel, W_root,
           W_fc, b_fc, Wq, bq, Wk, bk, Wv, bv, Wo, bo, W_mlp, b_mlp):
    params = dict(pos_table=pos_table, W_rel=W_rel, b_rel=b_rel, W_root=W_root,
                  W_fc=W_fc, b_fc=b_fc, Wq=Wq, bq=bq, Wk=Wk, bk=bk, Wv=Wv, bv=bv,
                  Wo=Wo, bo=bo, W_mlp=W_mlp, b_mlp=b_mlp)
    W = _prep_weights(params)
    A, shared = _build_shared_A(edge_src, edge_dst, edge_weight)
    if not shared:
        A = _build_A_all(edge_src, edge_dst, edge_weight)
    xw = _pack_x(X, y)
    try:
        import jax
        dev = jax.devices()[:M]
        dA, dW = _stage_constants(A, W, shared)
        pool = _get_pool()
        futs = [pool.submit(jax.device_put, xw[i], dev[i]) for i in range(M)]
        dxw = jax.device_put_sharded([f.result() for f in futs], dev)
        fn = _get_fn(shared)
        out = fn(dA, dxw, dW)                                   # [M,BL,T,N,HID] f16
        res = np.empty((M, BL, T, N, HID), np.float16)

        def _fetch(i, s):
            res[i] = np.asarray(s.data).reshape(BL, T, N, HID)

        shards = sorted(out.addressable_shards, key=lambda s: s.index[0].start or 0)
        list(pool.map(lambda a: _fetch(*a), enumerate(shards)))
        return res.reshape(B, T, N, HID).astype(np.float32)
    except Exception:
        return _cpu_fallback(xw, W, A, shared)


def _cpu_fallback(xw, W, A, shared):
    x = np.asarray(xw, np.float32).reshape(M, N, BL, T, C)
    x = x.transpose(0, 2, 1, 3, 4).reshape(B, N, T, C)          # [B,N,T,C]
    A = np.asarray(A, np.float32)
    if shared:
        xT = x.transpose(1, 0, 2, 3).reshape(N, B * T * C)
        agg = (A @ xT).reshape(N, B, T, C).transpose(1, 0, 2, 3)
    else:
        agg = np.einsum('bij,bjtc->bitc', A.reshape(B, N, N), x)
    sig = lambda a: 1.0 / (1.0 + np.exp(-a))
    h = sig(agg @ W['W_rel'] + x @ W['W_root'] + W['b_rel'])
    fc = h @ W['W_fc_h'] + x @ W['W_fc_x'] + W['bias_fc'][None, None, :, :]
    z = fc.reshape(B * N, T, HID)
    ctxs = []
    for hi in range(HEADS):
        sl = slice(hi * D_HEAD, (hi + 1) * D_HEAD)
        qh = z @ W['Wq'][:, sl] + W['bq'][sl]
        kh = z @ W['Wk'][:, sl] + W['bk'][sl]
        vh = z @ W['Wv'][:, sl] + W['bv'][sl]
        s = np.einsum('gqd,gkd->gqk', qh, kh)
        e = np.exp(s - s.max(-1, keepdims=True))
        a = e / e.sum(-1, keepdims=True)
        ctxs.append(np.einsum('gqk,gkd->gqd', a, vh))
    ctx = np.concatenate(ctxs, -1)
    o = ctx @ W['Wom'] + W['bom']
    return (o.reshape(B, N, T, HID).transpose(0, 2, 1, 3)).astype(np.float32)
